# revision 55
# baseline (speedup 1.0000x reference)
"""Trainium2 Bass kernel for nn_Attention_17016660426876.

Full-input contract: kernel(**inputs) takes the unsharded inputs and returns
the full (4, 2048, 1024) output. 8 NeuronCores: core c handles batch b=c//2,
head-half hh=c%2 (8 of 16 heads, as 4 head-pairs). Each core emits a partial
output projection; host sums core pairs.

Fused per-head-pair pipeline: while pair j's attention runs (ScalarE
exp-bound), the PE computes pair j+1's qkv projection + rmsnorm + rope in the
exp shadow; the output projection for token chunks runs in pair 3's shadow.
rmsnorm rstd uses Ln->Exp (exp(-0.5*ln(ms+eps))) so softmax exp and rstd share
one activation-table set (no table thrash).
"""

import sys

sys.path.insert(0, "/opt/trn_rl_repo")

from contextlib import ExitStack

import numpy as np

import concourse.bass as bass
import concourse.mybir as mybir
import concourse.tile as tile
from concourse import bacc
from concourse.bass_utils import run_bass_kernel_spmd

B, N, C, H, D = 4, 2048, 1024, 16, 64
NCORES = 8
HL = H // 2          # heads per core
CL = HL * D          # 512 local head-features
F_QK = 2 * CL        # q+k local features
EPS = 1e-6

F32 = mybir.dt.float32
F32R = mybir.dt.float32r
BF16 = mybir.dt.bfloat16
I16 = mybir.dt.int16

# Schraudolph bf16 exp: exp(x) ~= bitcast_bf16(i16(round(x*184.665 + b)));
# max rel err ~3.4%, cancels in softmax normalization. Used to offload a
# few key-chunks per quarter from ScalarE to Pool/DVE.
SCH_A = 184.6650390625
SCH_B = 16250.0
# GPSIMD cannot read PSUM on HW, so all offloaded exp chunks go to DVE;
# Pool instead takes the (SBUF-only) softmax-normalize multiplies.
POOL_KC = ()
DVE_KC = (15,)


def _fix_act_tables(arch):
    """Make the table-load pass pick natural_log_exp_and_others for both Ln
    and Exp (the greedy per-function choice would alternate exp_and_others /
    natural_log and reload tables on every switch). Mutates the cached
    name->funcset dict values in place; indices (= act_func_set_ids) are
    unchanged, so walrus still maps ids onto the real act_info.json."""
    from concourse.hw_specs import get_activation_tables
    ACT = mybir.ActivationFunctionType
    tabs = get_activation_tables(arch)
    both = "natural_log_exp_and_others"
    if both in tabs:
        for name, funcs in tabs.items():
            if name == both:
                continue
            funcs.discard(ACT.Exp)
            funcs.discard(ACT.Ln)


def build_nc():
    nc = bacc.Bacc("TRN2", target_bir_lowering=False, debug=False,
                   num_devices=NCORES)
    _fix_act_tables(nc.m.arch)

    xT = nc.dram_tensor("xT", [C, N], BF16, kind="ExternalInput")
    wq = nc.dram_tensor("wq", [8, 128, F_QK], BF16, kind="ExternalInput")
    wvT = nc.dram_tensor("wvT", [C, CL], BF16, kind="ExternalInput")
    wpT = nc.dram_tensor("wpT", [CL, C], BF16, kind="ExternalInput")
    tab_cq = nc.dram_tensor("tab_cq", [64, N], F32, kind="ExternalInput")
    tab_sq = nc.dram_tensor("tab_sq", [64, N], F32, kind="ExternalInput")
    tab_ck = nc.dram_tensor("tab_ck", [64, N], F32, kind="ExternalInput")
    tab_sk = nc.dram_tensor("tab_sk", [64, N], F32, kind="ExternalInput")
    rmat = nc.dram_tensor("rmat", [128, 128], F32R, kind="ExternalInput")
    blk2 = nc.dram_tensor("blk2", [128, 2], F32R, kind="ExternalInput")
    selqk4 = nc.dram_tensor("selqk4", [128, 128], F32R,
                            kind="ExternalInput")
    out = nc.dram_tensor("out", [N, C], BF16, kind="ExternalOutput")

    ACT = mybir.ActivationFunctionType

    with tile.TileContext(nc) as tc, ExitStack() as top:
        pers = top.enter_context(tc.tile_pool(name="pers", bufs=1))

        # ---- long-lived SBUF ----
        x_sb = [pers.tile([128, N], BF16, name=f"x{i}") for i in range(8)]
        tabs = {}
        for nm, dr in (("cq", tab_cq), ("sq", tab_sq),
                       ("ck", tab_ck), ("sk", tab_sk)):
            tabs[nm] = pers.tile([128, N], F32, name=f"tab_{nm}")
        # v with 64 ones-columns per head: (tok, head, 0:64=v, 64:128=1)
        v5 = [pers.tile([128, HL * 128], BF16, name=f"v5_{t}") for t in range(16)]
        aT = [pers.tile([128, N], BF16, name=f"aT{p}") for p in range(4)]
        rmat_sb = pers.tile([128, 128], F32R, name="rmat_sb")
        blk2_sb = pers.tile([128, 2], F32R, name="blk2_sb")
        selqk4_sb = pers.tile([128, 128], F32R, name="selqk4_sb")
        eps_sb = pers.tile([128, 1], F32, name="eps_sb")
        nc.vector.memset(eps_sb, EPS)
        # packed per-block mean-square staging: quarter qq's [2,512] ms rows
        # land at partitions {32qq, 32qq+1} so Ln/Exp cost free-512 (not
        # free-2048). Rows outside those groups stay at the 1.0 memset,
        # keeping Ln's input positive.
        msq_sb = [pers.tile([128, 512], F32, name=f"msq{i}")
                  for i in range(2)]
        for i in range(2):
            nc.vector.memset(msq_sb[i], 1.0)

        nc.sync.dma_start(out=rmat_sb, in_=rmat[:, :])
        nc.sync.dma_start(out=blk2_sb, in_=blk2[:, :])
        nc.sync.dma_start(out=selqk4_sb, in_=selqk4[:, :])

        # PE warm-up while input DMAs land (HAM at 2.4GHz needs ~3.4us busy);
        # uses a memset tile so it does not wait on any input DMA.
        warm_sb = pers.tile([128, 128], F32, name="warm_sb")
        nc.vector.memset(warm_sb, 0.0)
        with tc.tile_pool(name="warm", bufs=1, space="PSUM") as warmp:
            wps = warmp.tile([128, 128], F32, tag="warm", name="warm_ps")
            for i in range(48):
                nc.tensor.matmul(wps, warm_sb.bitcast(F32R),
                                 warm_sb.bitcast(F32R), start=True,
                                 stop=True)

        for i in range(8):
            nc.sync.dma_start(out=x_sb[i][:, 0:1024],
                              in_=xT[i * 128:(i + 1) * 128, 0:1024])

        # rotating SBUF pools
        qkTp = top.enter_context(tc.tile_pool(name="qkTp", bufs=4))
        wqp = top.enter_context(tc.tile_pool(name="wqp", bufs=2))
        rawp = top.enter_context(tc.tile_pool(name="rawp", bufs=2))
        sqp = top.enter_context(tc.tile_pool(name="sqp", bufs=1))
        uwp = top.enter_context(tc.tile_pool(name="uwp", bufs=2))
        v2p = top.enter_context(tc.tile_pool(name="v2p", bufs=4))
        msp = top.enter_context(tc.tile_pool(name="msp", bufs=1))

        qkT = {}  # (pair, 0=q/1=k) -> tile

        def emit_qk(p, rawpp, msrp, wq_pre=None, fine_rstd=False):
            """qkv projection + rmsnorm(via ln/exp) + rope for pair p.
            k-block first; raw psums and ms/rot/rb psums use separate pools
            so the qkv matmul stream decouples from the ms/rot round-trips."""
            wq_sb = {}
            for bi, blk in enumerate((p, p + 4)):
                if wq_pre is not None:
                    wq_sb[bi] = wq_pre[bi]
                else:
                    wt = wqp.tile([128, F_QK], BF16, tag="wq",
                                  name=f"wq{blk}")
                    nc.sync.dma_start(out=wt, in_=wq[blk])
                    wq_sb[bi] = wt
                qkT[(p, bi)] = qkTp.tile([128, N], BF16, tag="qkT",
                                         name=f"qkT{blk}")
            # k-block (bi=1) first so attention(p) can see k early; each
            # block carries its own rstd tail so the pair-boundary tail is
            # only the q-block's.
            for bi, blk in ((1, p + 4), (0, p)):
                v2s = {}
                for qq in range(4):
                    ts = slice(qq * 512, qq * 512 + 512)
                    raw_ps = rawpp.tile([128, 512], F32, tag="mr",
                                        name=f"rps{blk}_{qq}")
                    for ci in range(8):
                        nc.tensor.matmul(
                            raw_ps, wq_sb[bi][:, ci * 128:(ci + 1) * 128],
                            x_sb[ci][:, ts], start=(ci == 0),
                            stop=(ci == 7))
                    raw = rawp.tile([128, 512], F32R, tag="raw",
                                    name=f"raw{blk}_{qq}")
                    nc.vector.tensor_copy(out=raw, in_=raw_ps)
                    sq = sqp.tile([128, 512], F32R, tag="sq",
                                  name=f"sq{blk}_{qq}")
                    nc.gpsimd.tensor_mul(sq, raw.bitcast(F32),
                                         raw.bitcast(F32))
                    ms_ps = msrp.tile([128, 512], F32, tag="mm",
                                      name=f"mps{blk}_{qq}")
                    nc.tensor.matmul(ms_ps[0:2, :], blk2_sb, sq,
                                     start=True, stop=True)
                    if fine_rstd:
                        # ACT is idle during the head: rstd per quarter,
                        # straight off the PSUM ms, so qkT completes
                        # incrementally and attention can start sooner.
                        nc.scalar.activation(ms_ps[0:2, :], ms_ps[0:2, :],
                                             ACT.Ln, bias=eps_sb[0:2, :],
                                             scale=1.0)
                        rstd4 = msp.tile([2, 512], F32R, tag="rstd4",
                                         name=f"rstd4_{blk}_{qq}")
                        nc.scalar.activation(rstd4, ms_ps[0:2, :],
                                             ACT.Exp, scale=-0.5)
                    else:
                        # stage ms rows at partition group 32qq (upward
                        # partition shift) of the packed per-block tile
                        nc.vector.tensor_copy(
                            out=msq_sb[bi][qq * 32:qq * 32 + 2, :],
                            in_=ms_ps[0:2, :])
                    rot_ps = msrp.tile([128, 512], F32, tag="mm",
                                        name=f"rot{blk}_{qq}")
                    nc.tensor.matmul(rot_ps, rmat_sb, raw, start=True,
                                     stop=True)
                    tc_ = tabs["cq"] if bi == 0 else tabs["ck"]
                    tss = tabs["sq"] if bi == 0 else tabs["sk"]
                    u = uwp.tile([128, 512], F32, tag="u",
                                 name=f"u{blk}_{qq}")
                    nc.gpsimd.tensor_mul(u, raw.bitcast(F32), tc_[:, ts])
                    w = uwp.tile([128, 512], F32, tag="w",
                                 name=f"w{blk}_{qq}")
                    nc.vector.tensor_mul(w, rot_ps, tss[:, ts])
                    v2 = v2p.tile([128, 512], F32, tag="v2",
                                  name=f"v2_{blk}_{qq}")
                    nc.gpsimd.tensor_add(v2, u, w)
                    v2s[qq] = v2
                    if fine_rstd:
                        rb_ps = msrp.tile([128, 512], F32, tag="mm",
                                          name=f"rb{p}_{bi}_{qq}")
                        nc.tensor.matmul(rb_ps, selqk4_sb[0:2, :], rstd4,
                                         start=True, stop=True)
                        nc.vector.tensor_mul(qkT[(p, bi)][:, ts],
                                             v2, rb_ps)
                if not fine_rstd:
                    # rstd = exp(-0.5*ln(ms+eps)) on packed partition
                    # groups {0,32,64,96}: Ln/Exp cost free-512.
                    msq_ln = msp.tile([128, 512], F32, tag="msln",
                                      name=f"msln{blk}")
                    nc.scalar.activation(msq_ln, msq_sb[bi], ACT.Ln,
                                         bias=eps_sb, scale=1.0)
                    rstd_q = msp.tile([128, 512], F32R, tag="rstdq",
                                      name=f"rstdq{blk}")
                    nc.scalar.activation(rstd_q, msq_ln, ACT.Exp,
                                         scale=-0.5)
                    for qq in range(4):
                        ts = slice(qq * 512, qq * 512 + 512)
                        rb_ps = msrp.tile([128, 512], F32, tag="mm",
                                          name=f"rb{p}_{bi}_{qq}")
                        nc.tensor.matmul(
                            rb_ps, selqk4_sb[qq * 32:qq * 32 + 2, :],
                            rstd_q[qq * 32:qq * 32 + 2, :],
                            start=True, stop=True,
                            tile_position=(qq * 32, 0))
                        nc.vector.tensor_mul(qkT[(p, bi)][:, ts],
                                             v2s[qq], rb_ps)

        # ------- v-compute + qk(0) + attention(0), overlapped -------
        # wvp/vps live on the RIGHT allocation stacks so the long-lived
        # attention pools (left) can open while v(8:16) is still streaming;
        # attention(0) is emitted alongside v(8:16) and the simulation-driven
        # scheduler interleaves by readiness, starting the exp stream as
        # soon as qkT(0) is ready instead of after all of v.
        with ExitStack() as v_scope:
            wvp = v_scope.enter_context(
                tc.tile_pool(name="wvp", bufs=1, side="right"))
            wv_sb = [wvp.tile([128, CL], BF16, name=f"wv{i}")
                     for i in range(8)]
            for i in range(8):
                nc.sync.dma_start(out=wv_sb[i],
                                  in_=wvT[i * 128:(i + 1) * 128, :])
            for i in range(8):
                nc.sync.dma_start(out=x_sb[i][:, 1024:2048],
                                  in_=xT[i * 128:(i + 1) * 128, 1024:2048])
            for nm, dr in (("cq", tab_cq), ("sq", tab_sq),
                           ("ck", tab_ck), ("sk", tab_sk)):
                nc.sync.dma_start(out=tabs[nm][0:64, :], in_=dr[:, :])
                nc.sync.dma_start(out=tabs[nm][64:128, :], in_=dr[:, :])
            for t in range(16):
                nc.vector.memset(v5[t], 1.0)
            vps = v_scope.enter_context(
                tc.tile_pool(name="vps", bufs=2, space="PSUM",
                             side="right"))

            def emit_v(tks):
                for tk in tks:
                    ks = slice(tk * 128, tk * 128 + 128)
                    ps = vps.tile([128, CL], F32, tag="pv", name=f"pv{tk}")
                    for ci in range(8):
                        nc.tensor.matmul(ps, x_sb[ci][:, ks], wv_sb[ci],
                                         start=(ci == 0), stop=(ci == 7))
                    nc.vector.tensor_copy(
                        out=v5[tk].rearrange("p (h e) -> p h e",
                                             h=HL)[:, :, 0:64],
                        in_=ps.rearrange("p (h d) -> p h d", h=HL))

            emit_v(range(8))
            with ExitStack() as qk0_scope:
                rawp0 = qk0_scope.enter_context(
                    tc.tile_pool(name="rawp0", bufs=2, space="PSUM"))
                msrp0 = qk0_scope.enter_context(
                    tc.tile_pool(name="msrp0", bufs=2, space="PSUM"))
                emit_qk(0, rawp0, msrp0, fine_rstd=True)

            # attention-phase pools (left stacks; coexist with vps/wvp)
            sps = top.enter_context(
                tc.tile_pool(name="sps", bufs=2, space="PSUM"))
            opp = top.enter_context(
                tc.tile_pool(name="opp", bufs=1, space="PSUM"))
            ptp = top.enter_context(tc.tile_pool(name="ptp", bufs=6))
            rbap = top.enter_context(tc.tile_pool(name="rbap", bufs=1))
            outp = top.enter_context(tc.tile_pool(name="outp", bufs=2))

            def emit_attention(p, quarters, last=False):
                qt, kt = qkT[(p, 0)], qkT[(p, 1)]
                for qq in quarters:
                    qs = slice(qq * 512, qq * 512 + 512)
                    op = opp.tile([128, 1024], F32, tag="op",
                                  name=f"op{p}_{qq}")
                    for kc in range(16):
                        ks = slice(kc * 128, kc * 128 + 128)
                        sp = sps.tile([128, 1024], F32, tag="sp",
                                      name=f"sp{p}_{qq}_{kc}")
                        nc.tensor.matmul(sp[:, 0:512], kt[0:64, ks],
                                         qt[0:64, qs], start=True,
                                         stop=True)
                        nc.tensor.matmul(sp[:, 512:1024], kt[64:128, ks],
                                         qt[64:128, qs], start=True,
                                         stop=True, skip_group_check=True)
                        if kc in POOL_KC or kc in DVE_KC:
                            # Schraudolph exp on Pool/DVE to unload
                            # ScalarE (the inner-loop pacing engine)
                            pt_i = ptp.tile([128, 1024], I16,
                                            tag="pti", bufs=2,
                                            name=f"pt{p}_{qq}_{kc}")
                            eng = (nc.gpsimd if kc in POOL_KC
                                   else nc.vector)
                            eng.tensor_scalar(
                                pt_i, sp, SCH_A * 0.125, SCH_B,
                                mybir.AluOpType.mult,
                                mybir.AluOpType.add)
                            pt = pt_i.bitcast(BF16)
                        else:
                            pt = ptp.tile([128, 1024], BF16, tag="pt",
                                          bufs=4,
                                          name=f"pt{p}_{qq}_{kc}")
                            nc.scalar.activation(pt, sp, ACT.Exp,
                                                 scale=0.125)
                        for hi in range(2):
                            vsl = v5[kc].rearrange(
                                "p (h e) -> p h e", h=HL)[:, 2 * p + hi, :]
                            nc.tensor.matmul(
                                op[:, hi * 512:hi * 512 + 512], vsl,
                                pt[:, hi * 512:hi * 512 + 512],
                                start=(kc == 0), stop=(kc == 15),
                                skip_group_check=True)
                    # free `op` ASAP (PV of the next quarter reuses it):
                    # ONE wide copy to SBUF releases the PSUM bank in
                    # ~1.2us; the denominator rows are then re-staged
                    # from SBUF (partition down-shift copy) for the
                    # base-0-aligned recip+muls.
                    opc = rbap.tile([128, 1024], F32, tag="opc",
                                    name=f"opc{p}_{qq}")
                    rden = rbap.tile([64, 1024], F32, tag="rden",
                                     name=f"rden{p}_{qq}")
                    if last:
                        # final drain: ScalarE is idle after the last
                        # exp — stage numerator there while DVE pulls
                        # the denominator straight from PSUM.
                        nc.scalar.copy(out=opc, in_=op)
                        nc.vector.tensor_copy(out=rden,
                                              in_=op[64:128, :])
                    else:
                        nc.vector.tensor_copy(out=opc, in_=op)
                        nc.vector.tensor_copy(out=rden,
                                              in_=opc[64:128, :])
                    rba = rbap.tile([64, 1024], F32, tag="rba",
                                    name=f"rba{p}_{qq}")
                    nc.vector.reciprocal_approx_fast(out=rba, in_=rden)
                    for hi in range(2):
                        os_ = slice(hi * 512, hi * 512 + 512)
                        nc.vector.tensor_mul(
                            aT[p][hi * 64:hi * 64 + 64, qs],
                            opc[0:64, os_], rba[:, os_])

            emit_v(range(8, 16))
            emit_attention(0, (0, 1, 2, 3))

        # v pools (wvp/vps) released; wp tiles reuse that SBUF space
        wpp = top.enter_context(tc.tile_pool(name="wpp", bufs=1))
        wp_sb = [wpp.tile([128, C], BF16, name=f"wp{i}") for i in range(4)]
        for i in range(4):
            nc.sync.dma_start(out=wp_sb[i],
                              in_=wpT[i * 128:(i + 1) * 128, :])

        with ExitStack() as misc_scope:
            rawpm = misc_scope.enter_context(
                tc.tile_pool(name="rawpm", bufs=1, space="PSUM"))
            msrpm = misc_scope.enter_context(
                tc.tile_pool(name="msrpm", bufs=1, space="PSUM"))
            emit_qk(1, rawpm, msrpm)
            emit_attention(1, (0, 1, 2, 3))
            emit_qk(2, rawpm, msrpm)
            emit_attention(2, (0, 1, 2, 3))
            emit_qk(3, rawpm, msrpm)

        # ---- pair 3 attention interleaved with output projection ----
        # proj runs one quarter BEHIND attention so the PE never
        # heads-of-line blocks on quarter qq3's softmax-normalize.
        with tc.tile_pool(name="pjp", bufs=2, space="PSUM") as pjp:
            emit_attention(3, (0,))
            for qq3 in range(4):
                if qq3 < 3:
                    emit_attention(3, (qq3 + 1,), last=(qq3 == 2))
                for tk in range(4 * qq3, 4 * qq3 + 4):
                    ks = slice(tk * 128, tk * 128 + 128)
                    for oh in range(2):
                        os_ = slice(oh * 512, oh * 512 + 512)
                        pp = pjp.tile([128, 512], F32, tag="pp",
                                      name=f"pp{tk}_{oh}")
                        for ci in range(4):
                            nc.tensor.matmul(pp, aT[ci][:, ks],
                                             wp_sb[ci][:, os_],
                                             start=(ci == 0),
                                             stop=(ci == 3))
                        ot = outp.tile([128, 512], BF16, tag="ot",
                                       name=f"ot{tk}_{oh}")
                        nc.vector.tensor_copy(out=ot, in_=pp)
                        nc.sync.dma_start(out=out[ks, os_], in_=ot)

    nc.compile()
    return nc


def prep_inputs(x, cos, sin, w_qkv, w_proj, q_gamma, k_gamma):
    import ml_dtypes
    bf16 = ml_dtypes.bfloat16

    x = np.asarray(x, np.float32)
    cos = np.asarray(cos, np.float32)
    sin = np.asarray(sin, np.float32)
    w_qkv = np.asarray(w_qkv, np.float32)
    w_proj = np.asarray(w_proj, np.float32)
    q_gamma = np.asarray(q_gamma, np.float32)
    k_gamma = np.asarray(k_gamma, np.float32)

    cosT = np.ascontiguousarray(cos[0, 0].T)      # (64, N)
    sinT = np.ascontiguousarray(sin[0, 0].T)

    def tables(g):
        g_swap = g.reshape(D // 2, 2)[:, ::-1].reshape(D)
        ct = cosT * g[:, None]
        st = sinT * g_swap[:, None]
        return np.ascontiguousarray(ct), np.ascontiguousarray(st)

    cq_t, sq_t = tables(q_gamma)
    ck_t, sk_t = tables(k_gamma)

    rmat = np.zeros((128, 128), np.float32)
    idx = np.arange(0, 128, 2)
    rmat[idx, idx + 1] = 1.0
    rmat[idx + 1, idx] = -1.0

    blk2 = np.zeros((128, 2), np.float32)
    blk2[0:64, 0] = 1.0 / 64
    blk2[64:128, 1] = 1.0 / 64

    # selqk4: row 32*qq -> head-A partition mask, 32*qq+1 -> head-B
    selqk4 = np.zeros((128, 128), np.float32)
    for qq in range(4):
        selqk4[qq * 32, 0:64] = 1.0
        selqk4[qq * 32 + 1, 64:128] = 1.0

    in_maps = []
    for c in range(NCORES):
        b, hh = c // 2, c % 2
        xT = np.ascontiguousarray(x[b].T).astype(bf16)
        wq_rows = w_qkv[512 * hh:512 * hh + 512]
        wk_rows = w_qkv[1024 + 512 * hh:1024 + 512 * hh + 512]
        wv_rows = w_qkv[2048 + 512 * hh:2048 + 512 * hh + 512]
        wqkT = np.concatenate([wq_rows, wk_rows], 0).T   # (1024 c, 1024 f)
        wq_tiled = np.ascontiguousarray(
            wqkT.reshape(8, 128, 8, 128).transpose(2, 1, 0, 3)
            .reshape(8, 128, F_QK)).astype(bf16)
        wvT = np.ascontiguousarray(wv_rows.T).astype(bf16)   # (1024, 512)
        wpT = np.ascontiguousarray(
            w_proj[:, 512 * hh:512 * hh + 512].T).astype(bf16)
        in_maps.append({
            "xT": xT, "wq": wq_tiled, "wvT": wvT, "wpT": wpT,
            "tab_cq": cq_t, "tab_sq": sq_t, "tab_ck": ck_t, "tab_sk": sk_t,
            "rmat": rmat, "blk2": blk2, "selqk4": selqk4,
        })
    return in_maps


_NC_CACHE = None


def get_nc():
    global _NC_CACHE
    if _NC_CACHE is None:
        _NC_CACHE = build_nc()
    return _NC_CACHE


def kernel(x, cos, sin, w_qkv, w_proj, q_gamma, k_gamma):
    nc = get_nc()
    in_maps = prep_inputs(x, cos, sin, w_qkv, w_proj, q_gamma, k_gamma)
    res = run_bass_kernel_spmd(nc, in_maps, list(range(NCORES)))
    parts = [np.asarray(res.results[c]["out"], dtype=np.float32)
             for c in range(NCORES)]
    out = np.stack([parts[2 * b] + parts[2 * b + 1] for b in range(B)])
    return out.astype(np.float32)

